# revision 6
# baseline (speedup 1.0000x reference)
"""Delta-threshold encoder (DeltaModulator) Trainium2 Bass kernel.

Input  x: (2048, 128, 320) f32.
Output y: (2048, 128, 620) f32 = [UP flags (300) | DN flags (300) | tail (20)].

Per (batch,row) element the reference runs a 300-step serial scan:
    up_t = x_t > dc + d;  dn_t = x_t < dc - d;  dc <- x_t if (up|dn) else dc

Key identities (exact in fp32): a trigger always changes dc, so with
    c_t = sign(dc_t - dc_{t-1})  in {-1, 0, +1}
we have  up_t == (c_t > 0)  and  dn_t == (c_t < 0).  The device emits ONE
int8 plane of 300 c-values per element (300 B) instead of two f32 flag
planes (2400 B); the host widens c to the UP/DN f32 planes and copies the
20-float tail directly from the input it already holds.

Per-core device traffic: read 37.5 MB (x cols 0:300), write 9.8 MB (c).
DMA efficiency note: each contiguous run is one descriptor (~80 ns of one
of 16 DMA engines), so both DRAM and SBUF sides are laid out fully
contiguous per (group, chunk) transfer - the host pre-arranges x into
chunk-major [core, g, k, p, f, t] order (128 descriptors of 38.4 KB per
transfer instead of 16384 x 300 B).

Strategy (8 NeuronCores, batch-sharded, no communication):
  - 32768 elements per core as 2 groups x 128 partitions x 128 elements;
    time processed in 4 chunks of 75 columns.
  - The serial recurrence runs as one custom DVE instruction per time
    step over (128, 128) elements, the two groups' chains interleaved so
    dependent instructions are 2 apart (hides the DVE writeback latency
    that would otherwise add ~160 ns/step). The dc trace overwrites the
    x chunk in place (step tau writes slot tau); the incoming dc of each
    chunk lives in a tiny separate carry tile so the main buffer stays
    DMA-contiguous.
  - Flags: diff = dc_next - dc_prev in place over the trace (writes
    trail reads), then ACT converts sign(diff) to int8. The step-0 diff
    (against the carry) is a 1-column DVE op into the carry tile. The
    big per-chunk diffs are split Pool(7)/DVE(1) to balance engine load
    (Pool tensor_tensor runs at 0.42 efficiency).
  - DMA queues: group-0 traffic on the SP HWDGE queue, group-1 on the
    Activation HWDGE queue; 16 DMA engines serve each queue.
"""

import numpy as np

import concourse.bacc as bacc
import concourse.tile as tile
from concourse import mybir, dve_ops
from concourse.dve_spec import Spec, Src0, Src1, C0, C1, select, lower, _has_src1
from concourse.dve_uop import DveOpSpec
from concourse.bass_utils import run_bass_kernel_spmd

DELTA = 0.02
B, R, TIN = 2048, 128, 320
TSCAN, TTAIL = 300, 20
TOUT = TSCAN * 2 + TTAIL  # 620
NCORES = 8
G, P, F = 2, 128, 128     # groups x partitions x elems-per-partition per core
K, TC = 4, 75             # time chunks x columns per chunk (K*TC == TSCAN)


def _delta_step_op():
    """Register (once) the fused scan-step DVE op:
    out = select((in0 > in1 + s0) | (in0 < in1 + s1), in0, in1)."""
    name = "DELTA_STEP_ANT"
    for op in dve_ops.OPS:
        if op.name == name:
            return op
    up = Src0 > (Src1 + C0)
    dn = Src0 < (Src1 + C1)
    spec = Spec(
        body=select(up | dn, Src0, Src1),
        reference=lambda in0, in1, s0, s1, imm2: np.where(
            (in0 > in1 + s0) | (in0 < in1 + s1), in0, in1
        ).astype(np.float32),
    )
    row = dve_ops._CUSTOM_DVE_ROW_BASE + len(dve_ops.OPS)
    dve_ops._SUB_OPCODE_FOR_NAME[name] = row
    shas = {
        v: DveOpSpec(
            name=name, opcode=row, uops=lower(spec, ver=v), rd1_en=_has_src1(spec)
        ).sha(v)
        for v in ("v3", "v4")
    }
    op = dve_ops.DveOp(name, spec, subdim=False, uops_sha=shas)
    dve_ops.OPS.append(op)
    dve_ops.CUSTOM_DVE_SPECS[name] = spec
    return op


def _build_module():
    step_op = _delta_step_op()
    nc = bacc.Bacc(
        "TRN2",
        target_bir_lowering=False,
        debug=False,
        enable_asserts=False,
        num_devices=NCORES,
    )
    x = nc.dram_tensor("x", [G, K, P, F, TC], mybir.dt.float32,
                       kind="ExternalInput")
    c = nc.dram_tensor("c", [G, K, P, F, TC], mybir.dt.int8,
                       kind="ExternalOutput")

    sub = mybir.AluOpType.subtract
    Sign = mybir.ActivationFunctionType.Sign
    Copy = mybir.ActivationFunctionType.Copy
    in_q = {0: nc.sync, 1: nc.scalar}   # per-group DMA queues (in and out)

    with tile.TileContext(nc) as tc:
        with (
            tc.tile_pool(name="wbuf", bufs=4) as wpool,
            tc.tile_pool(name="cbuf", bufs=3) as cpool,
            tc.tile_pool(name="carrybuf", bufs=4) as rpool,
        ):
            w, ct, carry = {}, {}, {}

            def dma_in(g, k):
                w[g, k] = wpool.tile([P, F, TC], mybir.dt.float32, tag="w",
                                     name=f"w_{g}_{k}")
                in_q[g].dma_start(w[g, k][:], x[g, k])

            for g in range(G):
                dma_in(g, 0)
                carry[g, 0] = rpool.tile([P, F, 1], mybir.dt.float32,
                                         tag="r", name=f"r_{g}_0")
                nc.gpsimd.memset(carry[g, 0][:, :, 0], 0.0)

            for k in range(K):
                if k + 1 < K:
                    for g in range(G):
                        dma_in(g, k + 1)
                        carry[g, k + 1] = rpool.tile(
                            [P, F, 1], mybir.dt.float32, tag="r",
                            name=f"r_{g}_{k + 1}")
                # Serial scan, the two groups' chains interleaved on DVE.
                # Step tau: w[tau] <- select(trigger(w[tau], dc), w[tau], dc)
                # where dc = w[tau-1] (or the carry tile for tau == 0).
                for tau in range(TC):
                    for g in range(G):
                        nc.vector._custom_dve(
                            step_op,
                            out=w[g, k][:, :, tau],
                            in0=w[g, k][:, :, tau],
                            in1=(w[g, k][:, :, tau - 1] if tau > 0
                                 else carry[g, k][:, :, 0]),
                            s0=DELTA,
                            s1=-DELTA,
                        )
                for g in range(G):
                    # Save outgoing dc for the next chunk (ACT copy; slot
                    # TC-1 is never overwritten by the diffs below).
                    if k + 1 < K:
                        nc.scalar.activation(carry[g, k + 1][:, :, 0],
                                             w[g, k][:, :, TC - 1], Copy)
                    # Step-0 diff: dc_0 - carry, in place into the carry
                    # tile (DVE; must precede the main diff's overwrite
                    # of w[0] - the tile deps serialize that).
                    nc.vector.tensor_tensor(
                        carry[g, k][:, :, 0], w[g, k][:, :, 0],
                        carry[g, k][:, :, 0], sub)
                for g in range(G):
                    # Main diff for steps 1..TC-1, in place (writes trail
                    # reads): w[j] <- w[j+1] - w[j].  Split Pool/DVE for
                    # engine balance: DVE takes g1 of the last chunk.
                    eng = nc.vector if (k == K - 1 and g == 1) else nc.gpsimd
                    eng.tensor_tensor(
                        w[g, k][:, :, 0 : TC - 1], w[g, k][:, :, 1:TC],
                        w[g, k][:, :, 0 : TC - 1], sub)
                for g in range(G):
                    ct[g, k] = cpool.tile([P, F, TC], mybir.dt.int8, tag="c",
                                          name=f"c_{g}_{k}")
                    nc.scalar.activation(ct[g, k][:, :, 0],
                                         carry[g, k][:, :, 0], Sign)
                    nc.scalar.activation(ct[g, k][:, :, 1:TC],
                                         w[g, k][:, :, 0 : TC - 1], Sign)
                    in_q[g].dma_start(c[g, k], ct[g, k][:])
    nc.compile()
    return nc


_NC_CACHE = []


def _get_module():
    if not _NC_CACHE:
        _NC_CACHE.append(_build_module())
    return _NC_CACHE[0]


def _prepare_inputs(x: np.ndarray) -> list[dict]:
    """Full (B, R, 320) f32 -> per-core chunk-major [G, K, P, F, TC]."""
    xr = x.reshape(NCORES, G, P, F, TIN)[..., :TSCAN]
    xr = xr.reshape(NCORES, G, P, F, K, TC).transpose(0, 1, 4, 2, 3, 5)
    xc = np.ascontiguousarray(xr)
    return [{"x": xc[i]} for i in range(NCORES)]


def kernel(x: np.ndarray) -> np.ndarray:
    x = np.ascontiguousarray(np.asarray(x, dtype=np.float32))
    assert x.shape == (B, R, TIN)
    nc = _get_module()
    in_maps = _prepare_inputs(x)
    last_err = None
    for _ in range(3):  # transient device wedges recover on retry
        try:
            res = run_bass_kernel_spmd(nc, in_maps, core_ids=list(range(NCORES)))
            break
        except Exception as e:  # noqa: BLE001
            last_err = e
    else:
        raise last_err
    cs = np.stack([res.results[i]["c"] for i in range(NCORES)], axis=0)
    # (ncores, G, K, P, F, TC) -> (ncores, G, P, F, K, TC) -> (B, R, 300)
    cf = np.ascontiguousarray(cs.transpose(0, 1, 3, 4, 2, 5)).reshape(B, R, TSCAN)
    y = np.empty((B, R, TOUT), dtype=np.float32)
    y[:, :, 0:TSCAN] = cf > 0
    y[:, :, TSCAN : 2 * TSCAN] = cf < 0
    y[:, :, 2 * TSCAN :] = x[:, :, TSCAN:]
    return y


if __name__ == "__main__":
    rng = np.random.default_rng(0)
    xs = rng.standard_normal((B, R, TIN)).astype(np.float32)
    out = kernel(xs)
    print(out.shape, out.dtype)


# revision 10
# speedup vs baseline: 1.2651x; 1.2651x over previous
"""Delta-threshold encoder (DeltaModulator) Trainium2 Bass kernel.

Input  x: (2048, 128, 320) f32.
Output y: (2048, 128, 620) f32 = [UP flags (300) | DN flags (300) | tail (20)].

Per (batch,row) element the reference runs a 300-step serial scan:
    up_t = x_t > dc + d;  dn_t = x_t < dc - d;  dc <- x_t if (up|dn) else dc

Key identities (exact in fp32): a trigger always changes dc, so with
    c_t = sign(dc_t - dc_{t-1})  in {-1, 0, +1}
we have  up_t == (c_t > 0)  and  dn_t == (c_t < 0).  The device emits ONE
int8 plane of 300 c-values per element (300 B) instead of two f32 flag
planes (2400 B); the host widens c to the UP/DN f32 planes and copies the
20-float tail directly from the input it already holds.

Per-core device traffic: read 37.5 MB (x cols 0:300), write 9.8 MB (c).
DMA efficiency note: each contiguous run is one descriptor (~80 ns of one
of 16 DMA engines), so both DRAM and SBUF sides are laid out fully
contiguous per (group, chunk) transfer - the host pre-arranges x into
chunk-major [core, g, k, p, f, t] order (128 descriptors of 38.4 KB per
transfer instead of 16384 x 300 B).

Strategy (8 NeuronCores, batch-sharded, no communication):
  - 32768 elements per core as 2 groups x 128 partitions x 128 elements;
    time processed in 4 chunks of 75 columns.
  - The serial recurrence runs as one custom DVE instruction per time
    step over (128, 128) elements, the two groups' chains interleaved so
    dependent instructions are 2 apart (hides the DVE writeback latency
    that would otherwise add ~160 ns/step). The dc trace overwrites the
    x chunk in place (step tau writes slot tau); the incoming dc of each
    chunk lives in a tiny separate carry tile so the main buffer stays
    DMA-contiguous.
  - Flags: diff = dc_next - dc_prev in place over the trace (writes
    trail reads), then ACT converts sign(diff) to int8. The step-0 diff
    (against the carry) is a 1-column DVE op into the carry tile. The
    big per-chunk diffs are split Pool(7)/DVE(1) to balance engine load
    (Pool tensor_tensor runs at 0.42 efficiency).
  - DMA queues: group-0 traffic on the SP HWDGE queue, group-1 on the
    Activation HWDGE queue; 16 DMA engines serve each queue.
"""

import numpy as np

import concourse.bacc as bacc
import concourse.tile as tile
from concourse import mybir, dve_ops
from concourse.dve_spec import Spec, Src0, Src1, C0, C1, select, lower, _has_src1
from concourse.dve_uop import DveOpSpec
from concourse.bass_utils import run_bass_kernel_spmd

DELTA = 0.02
B, R, TIN = 2048, 128, 320
TSCAN, TTAIL = 300, 20
TOUT = TSCAN * 2 + TTAIL  # 620
NCORES = 8
G, P, F = 2, 128, 128     # groups x partitions x elems-per-partition per core
K, TC = 4, 75             # time chunks x columns per chunk (K*TC == TSCAN)


def _delta_step_op():
    """Register (once) the fused scan-step DVE op:
    out = select((in0 > in1 + s0) | (in0 < in1 + s1), in0, in1)."""
    name = "DELTA_STEP_ANT"
    for op in dve_ops.OPS:
        if op.name == name:
            return op
    up = Src0 > (Src1 + C0)
    dn = Src0 < (Src1 + C1)
    spec = Spec(
        body=select(up | dn, Src0, Src1),
        reference=lambda in0, in1, s0, s1, imm2: np.where(
            (in0 > in1 + s0) | (in0 < in1 + s1), in0, in1
        ).astype(np.float32),
    )
    row = dve_ops._CUSTOM_DVE_ROW_BASE + len(dve_ops.OPS)
    dve_ops._SUB_OPCODE_FOR_NAME[name] = row
    shas = {
        v: DveOpSpec(
            name=name, opcode=row, uops=lower(spec, ver=v), rd1_en=_has_src1(spec)
        ).sha(v)
        for v in ("v3", "v4")
    }
    op = dve_ops.DveOp(name, spec, subdim=False, uops_sha=shas)
    dve_ops.OPS.append(op)
    dve_ops.CUSTOM_DVE_SPECS[name] = spec
    return op


def _build_module():
    step_op = _delta_step_op()
    nc = bacc.Bacc(
        "TRN2",
        target_bir_lowering=False,
        debug=False,
        enable_asserts=False,
        num_devices=NCORES,
    )
    # Time-major free-dim layout (..., TC, F): each scan step's (128, F)
    # slice is then CONTIGUOUS in SBUF - strided slices throttle the
    # custom-DVE op to ~5 cycles/elem, contiguous ones stream.
    x = nc.dram_tensor("x", [G, K, P, TC, F], mybir.dt.float32,
                       kind="ExternalInput")
    c = nc.dram_tensor("c", [G, K, P, TC, F], mybir.dt.int8,
                       kind="ExternalOutput")

    sub = mybir.AluOpType.subtract
    Sign = mybir.ActivationFunctionType.Sign
    Copy = mybir.ActivationFunctionType.Copy
    in_q = {0: nc.sync, 1: nc.scalar}   # per-group DMA queues (in and out)

    with tile.TileContext(nc) as tc:
        with (
            tc.tile_pool(name="wbuf", bufs=4) as wpool,
            tc.tile_pool(name="cbuf", bufs=3) as cpool,
            tc.tile_pool(name="carrybuf", bufs=4) as rpool,
        ):
            w, ct, carry = {}, {}, {}

            def dma_in(g, k):
                w[g, k] = wpool.tile([P, TC, F], mybir.dt.float32, tag="w",
                                     name=f"w_{g}_{k}")
                in_q[g].dma_start(w[g, k][:], x[g, k])

            for g in range(G):
                dma_in(g, 0)
                carry[g, 0] = rpool.tile([P, 1, F], mybir.dt.float32,
                                         tag="r", name=f"r_{g}_0")
                nc.gpsimd.memset(carry[g, 0][:, 0, :], 0.0)

            for k in range(K):
                if k + 1 < K:
                    for g in range(G):
                        dma_in(g, k + 1)
                        carry[g, k + 1] = rpool.tile(
                            [P, 1, F], mybir.dt.float32, tag="r",
                            name=f"r_{g}_{k + 1}")
                # Serial scan, the two groups' chains interleaved on DVE.
                # Step tau: w[tau] <- select(trigger(w[tau], dc), w[tau], dc)
                # where dc = w[tau-1] (or the carry tile for tau == 0).
                for tau in range(TC):
                    for g in range(G):
                        nc.vector._custom_dve(
                            step_op,
                            out=w[g, k][:, tau, :],
                            in0=w[g, k][:, tau, :],
                            in1=(w[g, k][:, tau - 1, :] if tau > 0
                                 else carry[g, k][:, 0, :]),
                            s0=DELTA,
                            s1=-DELTA,
                        )
                for g in range(G):
                    # Save outgoing dc for the next chunk (ACT copy; row
                    # TC-1 is never overwritten by the diffs below).
                    if k + 1 < K:
                        nc.scalar.activation(carry[g, k + 1][:, 0, :],
                                             w[g, k][:, TC - 1, :], Copy)
                    # Step-0 diff: dc_0 - carry, in place into the carry
                    # tile (DVE; must precede the main diff's overwrite
                    # of w[0] - the tile deps serialize that).
                    nc.vector.tensor_tensor(
                        carry[g, k][:, 0, :], w[g, k][:, 0, :],
                        carry[g, k][:, 0, :], sub)
                for g in range(G):
                    # Main diff for steps 1..TC-1, in place (writes trail
                    # reads): w[j] <- w[j+1] - w[j].  Split Pool/DVE for
                    # engine balance: DVE takes g1 of the last chunk.
                    eng = nc.vector if (k == K - 1 and g == 1) else nc.gpsimd
                    eng.tensor_tensor(
                        w[g, k][:, 0 : TC - 1, :], w[g, k][:, 1:TC, :],
                        w[g, k][:, 0 : TC - 1, :], sub)
                for g in range(G):
                    ct[g, k] = cpool.tile([P, TC, F], mybir.dt.int8, tag="c",
                                          name=f"c_{g}_{k}")
                    nc.scalar.activation(ct[g, k][:, 0, :],
                                         carry[g, k][:, 0, :], Sign)
                    nc.scalar.activation(ct[g, k][:, 1:TC, :],
                                         w[g, k][:, 0 : TC - 1, :], Sign)
                    in_q[g].dma_start(c[g, k], ct[g, k][:])
    nc.compile()
    return nc


_NC_CACHE = []


def _get_module():
    if not _NC_CACHE:
        _NC_CACHE.append(_build_module())
    return _NC_CACHE[0]


def _prepare_inputs(x: np.ndarray) -> list[dict]:
    """Full (B, R, 320) f32 -> per-core chunk/time-major [G, K, P, TC, F]."""
    xr = x.reshape(NCORES, G, P, F, TIN)[..., :TSCAN]
    xr = xr.reshape(NCORES, G, P, F, K, TC).transpose(0, 1, 4, 2, 5, 3)
    xc = np.ascontiguousarray(xr)
    return [{"x": xc[i]} for i in range(NCORES)]


def kernel(x: np.ndarray) -> np.ndarray:
    x = np.ascontiguousarray(np.asarray(x, dtype=np.float32))
    assert x.shape == (B, R, TIN)
    nc = _get_module()
    in_maps = _prepare_inputs(x)
    last_err = None
    for _ in range(3):  # transient device wedges recover on retry
        try:
            res = run_bass_kernel_spmd(nc, in_maps, core_ids=list(range(NCORES)))
            break
        except Exception as e:  # noqa: BLE001
            last_err = e
    else:
        raise last_err
    cs = np.stack([res.results[i]["c"] for i in range(NCORES)], axis=0)
    # (ncores, G, K, P, TC, F) -> (ncores, G, P, F, K, TC) -> (B, R, 300)
    cf = np.ascontiguousarray(cs.transpose(0, 1, 3, 5, 2, 4)).reshape(B, R, TSCAN)
    y = np.empty((B, R, TOUT), dtype=np.float32)
    y[:, :, 0:TSCAN] = cf > 0
    y[:, :, TSCAN : 2 * TSCAN] = cf < 0
    y[:, :, 2 * TSCAN :] = x[:, :, TSCAN:]
    return y


if __name__ == "__main__":
    rng = np.random.default_rng(0)
    xs = rng.standard_normal((B, R, TIN)).astype(np.float32)
    out = kernel(xs)
    print(out.shape, out.dtype)
